# revision 1
# baseline (speedup 1.0000x reference)
"""Trainium2 Bass kernel for pointer-generator additive attention.

Full op (per batch b):
    dec_fea = s_t_hat @ W_d.T + b_d                         # (n,)
    att     = EF[b] + dec_fea[None,:] + cov[b][:,None]*W_c  # (t, n)
    score   = tanh(att) @ v                                 # (t,)
    attn    = renorm(softmax(score) * mask)                 # (t,)
    c_t     = attn @ EO[b]                                  # (n,)
    cov_next= cov + attn

Data-parallel over batch across 8 NeuronCores (8 batches/core, params
replicated, no collectives). HBM-bound: EF + EO = 64 MB/core (~180 us at
360 GB/s per core), measured ~250 us on silicon.

Per-core compute layout (all tensors stay in natural (t, n) layout):
  - W_d.T / s_t_hat.T are pre-transposed and bf16-cast on the host (pure
    input staging); dec_fea = s @ W_d.T + b_d is a tiny bf16 PE matmul.
  - EF tiles stream HBM->SBUF through gpsimd cast-DMAs (f32 -> bf16).
  - PE: att = I.T @ EF (identity matmul, EF is the moving operand, bf16
    1 cyc/row) accumulated in PSUM with one K=2 matmul adding
    [cov_b ; 1].T @ [W_c ; dec_fea[b]] (= cov (x) W_c + 1 (x) dec).
  - ScalarE: one tanh per (128 x 1024) PSUM tile -> bf16 SBUF.
  - VectorE: scalar_tensor_tensor (th * v_bcast) with accum_out gives the
    n-reduction (score) per t-tile as a PSUM-free column accumulator.
  - Scores regroup to a partition-0 row via one PE transpose + a small
    SBUF->SBUF DMA; masked softmax + renorm + coverage on 1-partition rows.
  - c_t: PE matmuls, lhsT = attn column (one PE transpose), rhs = natural
    bf16 EO tiles, accumulated over 8 t-tiles into (1 x 512) PSUM rows.
  - Software pipeline: phase C (c_t) lags phase A by one batch so the PE
    never waits on the softmax chain.
All heavy matmuls are bf16 (inputs are bf16-rounded; tolerance is 2e-2,
measured rel_err ~2.6e-3).
"""

import sys

if "/opt/trn_rl_repo" not in sys.path:
    sys.path.insert(0, "/opt/trn_rl_repo")

import ml_dtypes
import numpy as np

import concourse.bass as bass
import concourse.mybir as mybir
import concourse.tile as tile
from concourse import bacc
from concourse.bass_utils import run_bass_kernel_spmd
from concourse.masks import make_identity

F32 = mybir.dt.float32
BF16 = mybir.dt.bfloat16
AF = mybir.ActivationFunctionType
ALU = mybir.AluOpType
AX = mybir.AxisListType

N_CORES = 8
B = 64
NB = B // N_CORES  # local batches per core
T = 1024
N = 1024
TCH = 512          # t-chunk (one PSUM bank at fp32)
NBLK = N // 128    # n-blocks of 128 partitions
KT = N // 128      # k-tiles for the W_d matvec


def build_bass(nb: int = NB) -> bass.Bass:
    nc = bacc.Bacc()

    ef_d = nc.declare_dram_parameter("encoder_feature", [nb * T, N], F32, isOutput=False)
    eo_d = nc.declare_dram_parameter("encoder_outputs", [nb, T, N], F32, isOutput=False)
    mk_d = nc.declare_dram_parameter("enc_padding_mask", [nb, T], F32, isOutput=False)
    cv_d = nc.declare_dram_parameter("coverage", [nb, T], F32, isOutput=False)
    wdt_d = nc.declare_dram_parameter("W_d_T", [N, N], BF16, isOutput=False)
    st_d = nc.declare_dram_parameter("s_t_hat_T", [N, nb], BF16, isOutput=False)
    bd_d = nc.declare_dram_parameter("b_d", [N], BF16, isOutput=False)
    wc_d = nc.declare_dram_parameter("W_c", [N], BF16, isOutput=False)
    v_d = nc.declare_dram_parameter("v", [N], BF16, isOutput=False)
    ct_o = nc.declare_dram_parameter("c_t", [nb, N], F32, isOutput=True)
    at_o = nc.declare_dram_parameter("attn", [nb, T], F32, isOutput=True)
    cn_o = nc.declare_dram_parameter("coverage_next", [nb, T], F32, isOutput=True)

    with tile.TileContext(nc) as tc:
        with (
            tc.tile_pool(name="consts", bufs=1) as consts,
            tc.tile_pool(name="wdtp", bufs=1) as wdtp,
            tc.tile_pool(name="efp", bufs=8) as efp,
            tc.tile_pool(name="eop", bufs=16) as eop,
            tc.tile_pool(name="thp", bufs=6) as thp,
            tc.tile_pool(name="ttro", bufs=2) as ttro,
            tc.tile_pool(name="smal", bufs=4) as smal,
            tc.tile_pool(name="rowstg", bufs=2) as rowstg,
            tc.tile_pool(name="psA", bufs=2, space="PSUM") as psA,
            tc.tile_pool(name="psS", bufs=2, space="PSUM") as psS,
            tc.tile_pool(name="psT", bufs=2, space="PSUM") as psT,
        ):
            # ---------------- constants / small inputs ----------------
            ident = consts.tile([128, 128], F32)
            make_identity(nc, ident)
            ident_b = consts.tile([128, 128], BF16)
            nc.vector.tensor_copy(ident_b, ident)
            ones_f32 = consts.tile([1, T], F32)
            nc.vector.memset(ones_f32, 1.0)
            ones_b = consts.tile([1, T], BF16)
            nc.vector.tensor_copy(ones_b, ones_f32)

            bd_b = consts.tile([1, N], BF16)
            nc.sync.dma_start(out=bd_b, in_=bd_d[None, :])
            v_b = consts.tile([1, N], BF16)
            nc.sync.dma_start(out=v_b, in_=v_d[None, :])
            wc_b = consts.tile([1, N], BF16)
            nc.sync.dma_start(out=wc_b, in_=wc_d[None, :])

            sT_all = consts.tile([128, KT, 32], BF16)     # s_t_hat.T k-tiles
            wdt_all = wdtp.tile([128, KT, N], BF16)       # W_d.T k-tiles
            dec_rows = consts.tile([nb, N], BF16)         # dec_fea rows
            v_bcast = consts.tile([128, N], BF16)         # v broadcast to 128p

            # v broadcast: ones-column (x) v_row via K=1 matmuls
            for h in range(2):
                ps_vb = psT.tile([128, 512], F32, tag="tscratch")
                nc.tensor.matmul(
                    ps_vb, lhsT=ones_b[0:1, 0:128],
                    rhs=v_b[0:1, h * 512:(h + 1) * 512],
                    start=True, stop=True,
                )
                nc.scalar.activation(
                    v_bcast[:, h * 512:(h + 1) * 512], ps_vb, AF.Copy
                )

            # ---- W_d.T and s_t_hat.T come pre-transposed from the host ----
            for kj in range(KT):
                nc.sync.dma_start(
                    out=wdt_all[:, kj, :],
                    in_=wdt_d[kj * 128:(kj + 1) * 128, :],
                )
            nc.sync.dma_start(
                out=sT_all[:, :, 0:nb],
                in_=st_d.rearrange("(kj p) b -> p kj b", p=128),
            )

            # dec_fea rows = s_t_hat @ W_d.T + b_d   (bf16 matmuls, tiny)
            for h in range(2):
                sl = slice(h * 512, (h + 1) * 512)
                psd = psT.tile([nb, 512], F32, tag="tscratch")
                for kj in range(KT):
                    nc.tensor.matmul(
                        psd,
                        lhsT=sT_all[:, kj, 0:nb],
                        rhs=wdt_all[:, kj, sl],
                        start=(kj == 0), stop=False,
                    )
                nc.tensor.matmul(
                    psd, lhsT=ones_b[0:1, 0:nb], rhs=bd_b[0:1, sl],
                    start=False, stop=True,
                )
                nc.scalar.activation(dec_rows[:, sl], psd, AF.Copy)

            # ---------------- main loop over local batches ----------------
            attn_tiles = {}
            eo_tiles = {}

            def load_eo_tile(b, tj):
                eot = eop.tile([128, 1, N], BF16, tag="eo")
                nc.gpsimd.dma_start(
                    out=eot,
                    in_=eo_d[b, tj * 128:(tj + 1) * 128, :].rearrange(
                        "(i p) n -> p i n", p=128),
                )
                eo_tiles.setdefault(b, []).append(eot)

            def phase_a(b):
                # cov2 = [cov_b ; ones], rhs2 = [W_c ; dec_fea[b]]  (bf16)
                cov2 = rowstg.tile([2, T], BF16, tag="cov2")
                nc.gpsimd.dma_start(out=cov2[0:1, :], in_=cv_d[None, b, :])
                nc.sync.dma_start(out=cov2[1:2, :], in_=ones_b)
                rhs2 = rowstg.tile([2, N], BF16, tag="rhs2")
                nc.sync.dma_start(out=rhs2[0:1, :], in_=wc_b)
                nc.sync.dma_start(out=rhs2[1:2, :], in_=dec_rows[b:b + 1, :])
                cov_b = rowstg.tile([1, T], F32, tag="cov")
                nc.sync.dma_start(out=cov_b, in_=cv_d[None, b, :])
                mask_b = rowstg.tile([1, T], F32, tag="mask")
                nc.sync.dma_start(out=mask_b, in_=mk_d[None, b, :])

                score_cols = smal.tile([128, T // 128], F32, tag="scol")

                # att = EF + cov (x) W_c + 1 (x) dec ; tanh ; dot v
                for i in range(T // 128):
                    if True:
                        eft = efp.tile([128, 1, N], BF16, tag="ef")
                        r0 = b * T + i * 128
                        if b == 0:
                            # split first-batch loads so the first tile lands
                            # in ~6us instead of ~23us (per-queue bandwidth)
                            for q in range(4):
                                nc.gpsimd.dma_start(
                                    out=eft[q * 32:(q + 1) * 32, :, :],
                                    in_=ef_d[r0 + q * 32:r0 + (q + 1) * 32, :]
                                    .rearrange("(i p) n -> p i n", p=32),
                                )
                        else:
                            nc.gpsimd.dma_start(
                                out=eft,
                                in_=ef_d[r0:r0 + 128, :].rearrange(
                                    "(i p) n -> p i n", p=128),
                            )
                        if b > 0:
                            load_eo_tile(b - 1, i)
                        if b == nb - 1:
                            load_eo_tile(b, i)
                        ii = 0
                        att = psA.tile([128, N], F32, tag="att")
                        for h in range(2):
                            nc.tensor.matmul(
                                att[:, h * 512:(h + 1) * 512],
                                lhsT=ident_b,
                                rhs=eft[:, ii, h * 512:(h + 1) * 512],
                                start=True, stop=False, skip_group_check=True,
                            )
                        for h in range(2):
                            nc.tensor.matmul(
                                att[:, h * 512:(h + 1) * 512],
                                lhsT=cov2[:, i * 128:(i + 1) * 128],
                                rhs=rhs2[:, h * 512:(h + 1) * 512],
                                start=False, stop=True, skip_group_check=True,
                            )
                        th = thp.tile([128, N], BF16, tag="th")
                        nc.scalar.activation(th, att, AF.Tanh)
                        scr = ttro.tile([128, N], BF16, tag="ttro")
                        nc.vector.scalar_tensor_tensor(
                            out=scr, in0=th, scalar=1.0, in1=v_bcast,
                            op0=ALU.mult, op1=ALU.mult,
                            accum_out=score_cols[:, i:i + 1],
                        )

                # score columns -> one row via transpose + small sbuf-sbuf DMA
                ps8 = psT.tile([T // 128, 128], F32, tag="tscratch")
                nc.tensor.matmul(
                    ps8, lhsT=score_cols, rhs=ident, is_transpose=True,
                    start=True, stop=True,
                )
                score8 = smal.tile([T // 128, 128], F32, tag="s8")
                nc.scalar.activation(score8, ps8, AF.Copy)
                score_b = rowstg.tile([1, T], F32, tag="score")
                nc.sync.dma_start(
                    out=score_b[0:1, :].rearrange("p (j t) -> p j t", j=T // 128),
                    in_=score8,
                )

                # softmax + mask renorm + coverage update (partition-0 rows)
                attn_b = rowstg.tile([1, T], F32, tag="attn")
                covn_b = rowstg.tile([1, T], F32, tag="covn")
                # scores are O(1) (|s| < ~3): plain exp is safe, skip max-sub
                nc.scalar.activation(attn_b, score_b, AF.Exp, bias=0.0, scale=1.0)
                ssum = smal.tile([1, 1], F32, tag="s3")
                nc.vector.scalar_tensor_tensor(
                    out=attn_b, in0=attn_b, scalar=1.0, in1=mask_b,
                    op0=ALU.mult, op1=ALU.mult, accum_out=ssum,
                )
                rs = smal.tile([1, 1], F32, tag="s4")
                nc.vector.reciprocal(rs, ssum)
                nc.vector.tensor_scalar_mul(attn_b, attn_b, rs)
                nc.vector.tensor_add(covn_b, cov_b, attn_b)
                nc.sync.dma_start(out=at_o[None, b, :], in_=attn_b)
                nc.sync.dma_start(out=cn_o[None, b, :], in_=covn_b)

                # attn columns for phase C
                attn8 = smal.tile([T // 128, 128], F32, tag="a8")
                nc.sync.dma_start(
                    out=attn8,
                    in_=attn_b[0:1, :].rearrange("p (j t) -> p j t", j=T // 128),
                )
                acp = psT.tile([128, T // 128], F32, tag="tscratch")
                nc.tensor.matmul(
                    acp, lhsT=attn8, rhs=ident[0:T // 128, 0:T // 128],
                    is_transpose=True, start=True, stop=True,
                )
                acols = smal.tile([128, T // 128], BF16, tag="acols")
                nc.scalar.activation(acols, acp, AF.Copy)
                attn_tiles[b] = acols

            def phase_c(b):
                # c_t = attn @ EO[b] (EO tiles preloaded during phase A)
                acols = attn_tiles.pop(b)
                tiles = eo_tiles.pop(b)
                ct_b = rowstg.tile([1, N], F32, tag="ctb")
                ctps = [psS.tile([1, 512], F32, tag="srow", name=f"ctp{h}")
                        for h in range(N // 512)]
                for tj in range(T // 128):
                    if True:
                        eot = tiles[tj]
                        for h in range(N // 512):
                            nc.tensor.matmul(
                                ctps[h],
                                lhsT=acols[:, tj:tj + 1],
                                rhs=eot[:, 0, h * 512:(h + 1) * 512],
                                start=(tj == 0), stop=(tj == T // 128 - 1),
                                skip_group_check=True,
                            )
                for h in range(N // 512):
                    nc.scalar.activation(
                        ct_b[0:1, h * 512:(h + 1) * 512], ctps[h], AF.Copy
                    )
                nc.sync.dma_start(out=ct_o[None, b, :], in_=ct_b)

            # software pipeline: phase C lags one batch behind phase A
            for b in range(nb):
                phase_a(b)
                if b > 0:
                    phase_c(b - 1)
            phase_c(nb - 1)

    nc.finalize()
    return nc


_CACHE: dict = {}


def _get_nc() -> bass.Bass:
    if "nc" not in _CACHE:
        _CACHE["nc"] = build_bass(NB)
    return _CACHE["nc"]


def make_in_maps(inputs: dict) -> list:
    f = lambda x: np.ascontiguousarray(np.asarray(x), dtype=np.float32)
    s = f(inputs["s_t_hat"])
    eo = f(inputs["encoder_outputs"])
    ef = f(inputs["encoder_feature"]).reshape(B, T, N)
    mk = f(inputs["enc_padding_mask"])
    cv = f(inputs["coverage"])
    wdt = np.ascontiguousarray(f(inputs["W_d"]).T).astype(ml_dtypes.bfloat16)
    bd = f(inputs["b_d"])
    wc = f(inputs["W_c"])
    vv = f(inputs["v"])
    in_maps = []
    for i in range(N_CORES):
        sl = slice(i * NB, (i + 1) * NB)
        in_maps.append({
            "encoder_feature": np.ascontiguousarray(ef[sl]).reshape(NB * T, N),
            "encoder_outputs": eo[sl],
            "s_t_hat_T": np.ascontiguousarray(s[sl].T).astype(ml_dtypes.bfloat16),
            "enc_padding_mask": mk[sl],
            "coverage": cv[sl],
            "W_d_T": wdt,
            "b_d": bd.astype(ml_dtypes.bfloat16),
            "W_c": wc.astype(ml_dtypes.bfloat16),
            "v": vv.astype(ml_dtypes.bfloat16),
        })
    return in_maps


def gather_outputs(results: list):
    c_t = np.concatenate([results[i]["c_t"] for i in range(N_CORES)], axis=0)
    attn = np.concatenate([results[i]["attn"] for i in range(N_CORES)], axis=0)
    covn = np.concatenate(
        [results[i]["coverage_next"] for i in range(N_CORES)], axis=0
    )
    return c_t, attn, covn


def kernel(**inputs):
    nc = _get_nc()
    in_maps = make_in_maps(inputs)
    res = run_bass_kernel_spmd(nc, in_maps, core_ids=list(range(N_CORES)))
    return gather_outputs(res.results)

